# revision 36
# baseline (speedup 1.0000x reference)
"""Multi-head attention kernel for Trainium2 (Bass/Tile), 8 NeuronCores.

Problem: B=2, N=2048, C=512, H=8 heads, D=64. softmax(Q K^T / sqrt(D)) V.

Sharding: the 16 (batch, head) pairs are split 2-per-core across 8 cores
(data + head parallel, no communication).

Design (v2; ~74us/rep vs the ~104us v1 baseline):

  - All matmuls in bf16.  Q, K, V are loaded from HBM with gpsimd
    cast-DMAs (fp32 -> bf16 conversion is free in the DMA datapath) into
    [128, NT, 128] staging tiles whose columns 64..127 are zeroed once;
    a single DMA-xbar transpose (`dma_start_transpose`, 16x128 tiles)
    per tensor then produces Q^T / K^T in the canonical
    [d(64)+zero-pad(64), (t, p)] layout the PE wants — no DVE
    stream-transposes or block-permute DMAs in the prologue, and the
    128-partition zero pad (full SBUF->PE stream bandwidth for the
    moving operand) falls out of the xbar's row = ell % 128 mapping.

  - Per (b, h) pair ("slot"), per q-half (1024 queries), per k-chunk kc
    (16 chunks of 128 keys):
      ST[kc] = K_T[kc].T @ Q_T_half  -> [128k, 1024q] fp32 PSUM
      expST:  most (kc, qh) tiles on ScalarE (activation Exp, bf16 out);
      the 8 DVE_TILES run on the Vector engine as ONE tensor_scalar:
      i16 = round(ST*C1 + C2) written through an int16 bitcast of the
      bf16 ex tile IS the bf16 bit pattern of exp(ST*scale) (Schraudolph;
      convert-on-write rounds to nearest; +-3% scallop, tuned sigma).
      This splits exp across two engines (ScalarE alone would be the
      bottleneck at ~73us/rep).
      OT~ [65, 1024q] += [V[kc] | 1].T @ expST  (row 64 = denominator).
    PSUM: ST triple-buffered (3 x 2 banks) + one OT (2 banks) = 8 banks.
  - Epilogue per half: DVE 32x32 stream-transpose of OT~ out of PSUM,
    then (lagged, as fills inside later compute) a reciprocal of the
    denominator positions, two tiny partition-replication DMAs, and two
    32-partition broadcast multiplies — all IN the block-transposed
    layout.  The half is stored to HBM in that layout and
    unshard_output applies the inverse 32x32 block permute on the host,
    eliminating the 24 block-permute DMAs per slot that previously
    congested the sync/gpsimd queues and stalled the DVE stream.

  - Schedule: a flat stream of slots, 16 per unrolled For_i body
    (= 8 reps) plus a peeled first body; reps==1 is a 2-slot
    straight-line version of the same stream.  Slot j emits:
      loads(j+2)   [gpsimd cast-DMAs]
      xbars(j+1)   [sync HWDGE, async on the DMA engines]
      epi-A(j-1, half1)  [must precede slot j's first PV: reads the
                          shared OT PSUM banks; ready at the boundary]
      compute(j)   [QK -> exp -> lagged PV per kc; epi-A(j, half0)
                    inline at the half boundary; normalize+store of
                    older halves emitted as fills at kc2/kc6 so the
                    in-order DVE stream reaches each epilogue op long
                    after its inputs landed]
    Cross-slot tiles are static with an explicit j%2 buffer dim, so the
    lookahead survives the For_i back edge (the all-engine barrier costs
    ~16us once per 8 reps).  ScalarE executes nothing but ACTIVATEs —
    no DMA is ever issued on its queue — and the exp/PV/QK chain is
    decoupled by the ST triple-buffer so the PE streams matmuls
    back-to-back at ~216ns/512-col MM.
"""

import math
import sys

for _p in ("/opt/trn_rl_repo",):
    if _p not in sys.path:
        sys.path.insert(0, _p)

import numpy as np

import concourse.bass as bass  # noqa: F401
import concourse.bacc as bacc
import concourse.tile as tile
from concourse import mybir
from concourse.bass_utils import run_bass_kernel_spmd

F32 = mybir.dt.float32
BF16 = mybir.dt.bfloat16
I16 = mybir.dt.int16

B, N, C = 2, 2048, 512
H = 8
D = C // H           # 64
SCALE = float(D) ** -0.5
NT = N // 128        # 16 k-chunks / q-blocks
PAIRS = (B * H) // 8  # 2 (b,h) pairs per core
N_CORES = 8

# (kc, qh) exp tiles that run on the Vector engine (fast-exp) instead of
# ScalarE.  Spread over interior kcs and BOTH q-halves (each q row sees
# 4/16 of its k-terms approximated), alternating engines on each ST
# buffer so neither engine idles across a whole kc.  ScalarE keeps
# 32 - len(DVE_TILES) tiles (pace ~1.17us each vs the PE's ~1.73us/kc).
DVE_TILES = (
    (2, 0), (5, 0), (8, 0), (11, 0), (14, 0),
    (3, 1), (6, 1), (9, 1), (12, 1), (13, 1),
)

# Schraudolph constants: i16 = round(ST*C1 + C2) is the bf16 bit pattern
# of exp(ST*SCALE).  C2's sigma term centers the relative error of the
# linear-in-mantissa approximation (+-3% scallop).
EXP_C1 = float(SCALE * math.log2(math.e) * 128.0)
EXP_C2 = float((127.0 - 0.043) * 128.0)


def build_nc(reps=1, sim_safe=False):
    nc = bacc.Bacc()
    q_in = nc.dram_tensor("q_in", [PAIRS, N, D], F32, kind="ExternalInput")
    k_in = nc.dram_tensor("k_in", [PAIRS, N, D], F32, kind="ExternalInput")
    v_in = nc.dram_tensor("v_in", [PAIRS, N, D], F32, kind="ExternalInput")
    # Output is stored in the DVE 32x32 block-transposed layout
    # out[32*bb + r, t*128 + 32*a + c] = O[q = t*128 + 32*a + r, d = 32*bb + c];
    # unshard_output applies the inverse permute on the host.
    out_t = nc.dram_tensor("out", [PAIRS, D, N], F32, kind="ExternalOutput")

    ins = {"q": q_in, "k": k_in, "v": v_in}

    with tile.TileContext(nc) as tc:
        with (
            tc.tile_pool(name="stat", bufs=1) as stat,
            tc.tile_pool(name="ex", bufs=6) as ex_pool,
            tc.tile_pool(name="st", bufs=3, space="PSUM") as st_pool,
            tc.tile_pool(name="op", bufs=1, space="PSUM") as o_pool,
        ):
            # ---- static cross-slot tiles (explicit buf dim, index j%2) ----
            qstage = stat.tile([128, 2, NT, 128], BF16, tag="qstage")
            kstage = stat.tile([128, 2, NT, 128], BF16, tag="kstage")
            qt = stat.tile([128, 2, NT, 128], BF16, tag="qt")
            kt = stat.tile([128, 2, NT, 128], BF16, tag="kt")
            vt = stat.tile([128, 2, NT, D + 1], BF16, tag="vt")
            ot_tr = stat.tile([96, 2, N], F32, tag="ot_tr")
            o_nm = stat.tile([64, 2, N], F32, tag="o_nm")
            invx = stat.tile([96, 2, 2, 8, 4], F32, tag="invx")

            # one-time init: zero pad columns (never rewritten by loads),
            # ones column of V (denominator accumulator via PV matmul).
            # all on DVE: the gpsimd queue must start the first loads
            # immediately (gpsimd memset is slow and would delay them).
            nc.vector.memset(qstage[:, :, :, D:128], 0.0)
            nc.vector.memset(kstage[:, :, :, D:128], 0.0)
            nc.vector.memset(vt[:, :, :, D : D + 1], 1.0)

            # Warm the ScalarE Exp spline tables before any real work.
            warm = stat.tile([128, 1], F32, tag="warm")
            nc.vector.memset(warm[:], 0.0)
            nc.scalar.activation(
                warm[:], warm[:], mybir.ActivationFunctionType.Exp
            )

            def loads(j):
                b, pair = j % 2, j % PAIRS
                for name, t in ins.items():
                    src = t[pair].rearrange("(t p) d -> p t d", p=128)
                    if name == "v":
                        dst = vt[:, b, :, 0:D]
                    elif name == "q":
                        dst = qstage[:, b, :, 0:D]
                    else:
                        dst = kstage[:, b, :, 0:D]
                    nc.gpsimd.dma_start(out=dst, in_=src)

            def xbars(j):
                b = j % 2
                nc.sync.dma_start_transpose(qt[:, b], qstage[:, b])
                nc.sync.dma_start_transpose(kt[:, b], kstage[:, b])

            # OT~ accumulator: one static [96, 1024] PSUM tile (2 banks)
            # reused by both q-halves of every slot; ST triple-buffer takes
            # the other 6 banks.  A half's first PV (start=True) has a WAR
            # dep on the previous half's epilogue transpose reads.
            ot_ps = o_pool.tile([96, 1024], F32, tag="ot")
            if sim_safe:
                nc.vector.memset(ot_ps[D : 96, :], 0.0)

            def epi_a_half(j, qh):
                # OT~ half -> block-transposed staging ot_tr (DVE 32x32
                # stream transpose straight out of PSUM).  No permute DMAs:
                # normalization and the store happen in this layout; the
                # host-side unshard applies the inverse block permute.
                b = j % 2
                # split in two 512-col pieces: the next half's first PV
                # (start=True, cols 0:512) only has a WAR on the first
                # piece, so it unblocks ~0.6us earlier at each boundary.
                for hh in range(2):
                    nc.vector.transpose(
                        ot_tr[
                            :, b, qh * 1024 + hh * 512 : qh * 1024 + hh * 512 + 512
                        ],
                        ot_ps[:, hh * 512 : hh * 512 + 512],
                    )

            def epi_n1(j, qh):
                # reciprocal of the denominator (OT row 64 = ot_tr block
                # row 64..95, inner col 0) and partition-replication of the
                # result to bases 0 and 32 for the two d-block muls.
                b = j % 2
                otv = ot_tr[:, b, qh * 1024 : qh * 1024 + 1024].rearrange(
                    "p (t a c) -> p t a c", t=8, a=4
                )
                nc.vector.reciprocal(
                    invx[64:96, b, qh], otv[64:96, :, :, 0]
                )
                nc.sync.dma_start(out=invx[0:32, b, qh], in_=invx[64:96, b, qh])
                nc.sync.dma_start(out=invx[32:64, b, qh], in_=invx[64:96, b, qh])

            def epi_n2(j, qh):
                # normalize the two real d-blocks (rows 0..63; block row 2
                # holds only the denominator + garbage) and store the half
                # in block-transposed layout.
                b = j % 2
                cols = slice(qh * 1024, qh * 1024 + 1024)
                otv = ot_tr[:, b, cols].rearrange(
                    "p (t a c) -> p t a c", t=8, a=4
                )
                onv = o_nm[:, b, cols].rearrange(
                    "p (t a c) -> p t a c", t=8, a=4
                )
                nc.vector.tensor_mul(
                    onv[:],
                    otv[0:64],
                    invx[0:64, b, qh, :, :, None].broadcast_to([64, 8, 4, 32]),
                )
                nc.sync.dma_start(
                    out=out_t[j % PAIRS][:, cols], in_=o_nm[:, b, cols]
                )

            def compute(j, fills=None):
                b = j % 2
                qtv = qt[:, b].rearrange("d t p -> d (t p)")

                for qh in range(2):
                    def emit_pv(kc, ex):
                        for jj in range(2):
                            nc.tensor.matmul(
                                ot_ps[0 : D + 1, jj * 512 : jj * 512 + 512],
                                vt[:, b, kc, :],
                                ex[:, jj * 512 : jj * 512 + 512],
                                start=(kc == 0),
                                stop=(kc == NT - 1),
                            )

                    prevs = []
                    for kc in range(NT):
                        ex = ex_pool.tile([128, 1024], BF16, tag="ex")
                        st = st_pool.tile([128, 1024], F32, tag="st")
                        for jj in range(2):
                            q0 = qh * 1024 + jj * 512
                            nc.tensor.matmul(
                                st[:, jj * 512 : jj * 512 + 512],
                                kt[:, b, kc, :],
                                qtv[:, q0 : q0 + 512],
                                start=True,
                                stop=True,
                            )
                        if (kc, qh) in DVE_TILES:
                            nc.vector.tensor_scalar(
                                out=ex.bitcast(I16)[:],
                                in0=st[:],
                                scalar1=EXP_C1,
                                scalar2=EXP_C2,
                                op0=mybir.AluOpType.mult,
                                op1=mybir.AluOpType.add,
                            )
                        else:
                            nc.scalar.activation(
                                ex[:],
                                st[:],
                                mybir.ActivationFunctionType.Exp,
                                scale=SCALE,
                            )
                        if fills and (qh, kc) in fills:
                            for fn in fills[(qh, kc)]:
                                fn()
                        prevs.append(ex)
                        # PV lags exp by TWO kcs so its moving operand is
                        # always two exp-service periods old — the PE never
                        # waits out an exp tail + semaphore propagation.
                        if kc >= 2:
                            emit_pv(kc - 2, prevs[kc - 2])
                    emit_pv(NT - 2, prevs[NT - 2])
                    emit_pv(NT - 1, prevs[NT - 1])
                    if qh == 0:
                        epi_a_half(j, 0)

            def emit_slot(j, S, wrap, do_epi_a=True, do_epi_b=True):
                """Emit slot j of a stream of S slots. wrap=True means
                out-of-range lookahead/lag indices wrap mod S (loop body);
                wrap=False skips out-of-range work (straight-line path and
                the peeled first body via do_epi_a/do_epi_b)."""
                jl = (j + 2) % S if wrap else j + 2
                jx = (j + 1) % S if wrap else j + 1
                if wrap or jl < S:
                    loads(jl)
                if wrap or jx < S:
                    xbars(jx)
                # epi-A(j-1, half1) must run before compute(j)'s first PV
                # (it reads the shared OT PSUM banks); its inputs are ready
                # at the slot boundary so it doesn't stall the DVE stream.
                if do_epi_a and (wrap or j - 1 >= 0):
                    epi_a_half((j - 1) % S, 1)
                # The normalize+store of each transposed half rides as fills
                # a few kcs later, where the in-order DVE stream reaches it
                # with all inputs long since ready.
                fills = {}
                if do_epi_b and (wrap or j - 1 >= 0):
                    jp = (j - 1) % S
                    fills[(0, 2)] = [lambda: epi_n1(jp, 1)]
                    fills[(0, 6)] = [lambda: epi_n2(jp, 1)]
                fills[(1, 2)] = [lambda: epi_n1(j % S, 0)]
                fills[(1, 6)] = [lambda: epi_n2(j % S, 0)]
                compute(j, fills=fills)

            if reps == 1:
                S = PAIRS
                loads(0)
                loads(1)
                xbars(0)
                emit_slot(0, S, wrap=False)
                emit_slot(1, S, wrap=False)
                epi_a_half(1, 1)
                epi_n1(1, 1)
                epi_n2(1, 1)
            else:
                assert reps % 8 == 0, "reps must be 1 or a multiple of 8"
                S = 8 * PAIRS  # 16 slots per body
                loads(0)
                loads(1)
                xbars(0)
                # first body is peeled: its slots 0/1 have no prior slot
                # to lag behind, and its tail slots prefetch the loop's
                # first iteration.  For_i runs the rest.
                for j in range(S):
                    emit_slot(j, S, wrap=True,
                              do_epi_a=(j >= 1), do_epi_b=(j >= 2))
                with tc.For_i(0, reps // 8 - 1, 1):
                    for j in range(S):
                        emit_slot(j, S, wrap=True)
                epi_a_half(S - 1, 1)
                epi_n1(S - 1, 1)
                epi_n2(S - 1, 1)

    nc.compile()
    return nc


def shard_inputs(query, key, value):
    """[B, N, C] -> per-core dicts of [PAIRS, N, D] slices."""
    def to_pairs(x):
        return np.ascontiguousarray(
            x.reshape(B, N, H, D).transpose(0, 2, 1, 3).reshape(B * H, N, D)
        )

    qp, kp, vp = to_pairs(query), to_pairs(key), to_pairs(value)
    in_maps = []
    for c in range(N_CORES):
        s = slice(c * PAIRS, (c + 1) * PAIRS)
        in_maps.append({"q_in": qp[s], "k_in": kp[s], "v_in": vp[s]})
    return in_maps


def unshard_output(results):
    """per-core block-transposed [PAIRS, D, N] -> [B, N, C].

    Device layout: out[pair, 32*bb + r, t*128 + 32*a + c] =
    O[pair, q = t*128 + 32*a + r, d = 32*bb + c].
    """
    outs = np.concatenate([results[c]["out"] for c in range(N_CORES)], axis=0)
    arr = outs.reshape(B * H, 2, 32, NT, 4, 32)       # pair, bb, r, t, a, c
    o = arr.transpose(0, 3, 4, 2, 1, 5).reshape(B * H, N, D)  # pair, q, d
    return np.ascontiguousarray(
        o.reshape(B, H, N, D).transpose(0, 2, 1, 3).reshape(B, N, C)
    )


def kernel(query, key, value):
    query = np.asarray(query, dtype=np.float32)
    key = np.asarray(key, dtype=np.float32)
    value = np.asarray(value, dtype=np.float32)
    nc = build_nc()
    in_maps = shard_inputs(query, key, value)
    res = run_bass_kernel_spmd(nc, in_maps, core_ids=list(range(N_CORES)))
    return unshard_output(res.results)


# revision 38
# speedup vs baseline: 1.0144x; 1.0144x over previous
"""Multi-head attention kernel for Trainium2 (Bass/Tile), 8 NeuronCores.

Problem: B=2, N=2048, C=512, H=8 heads, D=64. softmax(Q K^T / sqrt(D)) V.

Sharding: the 16 (batch, head) pairs are split 2-per-core across 8 cores
(data + head parallel, no communication).

Design (v2; ~74us/rep vs the ~104us v1 baseline):

  - All matmuls in bf16.  Q, K, V are loaded from HBM with gpsimd
    cast-DMAs (fp32 -> bf16 conversion is free in the DMA datapath) into
    [128, NT, 128] staging tiles whose columns 64..127 are zeroed once;
    a single DMA-xbar transpose (`dma_start_transpose`, 16x128 tiles)
    per tensor then produces Q^T / K^T in the canonical
    [d(64)+zero-pad(64), (t, p)] layout the PE wants — no DVE
    stream-transposes or block-permute DMAs in the prologue, and the
    128-partition zero pad (full SBUF->PE stream bandwidth for the
    moving operand) falls out of the xbar's row = ell % 128 mapping.

  - Per (b, h) pair ("slot"), per q-half (1024 queries), per k-chunk kc
    (16 chunks of 128 keys):
      ST[kc] = K_T[kc].T @ Q_T_half  -> [128k, 1024q] fp32 PSUM
      expST:  most (kc, qh) tiles on ScalarE (activation Exp, bf16 out);
      the 8 DVE_TILES run on the Vector engine as ONE tensor_scalar:
      i16 = round(ST*C1 + C2) written through an int16 bitcast of the
      bf16 ex tile IS the bf16 bit pattern of exp(ST*scale) (Schraudolph;
      convert-on-write rounds to nearest; +-3% scallop, tuned sigma).
      This splits exp across two engines (ScalarE alone would be the
      bottleneck at ~73us/rep).
      OT~ [65, 1024q] += [V[kc] | 1].T @ expST  (row 64 = denominator).
    PSUM: ST triple-buffered (3 x 2 banks) + one OT (2 banks) = 8 banks.
  - Epilogue per half: DVE 32x32 stream-transpose of OT~ out of PSUM,
    then (lagged, as fills inside later compute) a reciprocal of the
    denominator positions, two tiny partition-replication DMAs, and two
    32-partition broadcast multiplies — all IN the block-transposed
    layout.  The half is stored to HBM in that layout and
    unshard_output applies the inverse 32x32 block permute on the host,
    eliminating the 24 block-permute DMAs per slot that previously
    congested the sync/gpsimd queues and stalled the DVE stream.

  - Schedule: a flat stream of slots, 16 per unrolled For_i body
    (= 8 reps) plus a peeled first body; reps==1 is a 2-slot
    straight-line version of the same stream.  Slot j emits:
      loads(j+2)   [gpsimd cast-DMAs]
      xbars(j+1)   [sync HWDGE, async on the DMA engines]
      epi-A(j-1, half1)  [must precede slot j's first PV: reads the
                          shared OT PSUM banks; ready at the boundary]
      compute(j)   [QK -> exp -> lagged PV per kc; epi-A(j, half0)
                    inline at the half boundary; normalize+store of
                    older halves emitted as fills at kc2/kc6 so the
                    in-order DVE stream reaches each epilogue op long
                    after its inputs landed]
    Cross-slot tiles are static with an explicit j%2 buffer dim, so the
    lookahead survives the For_i back edge (the all-engine barrier costs
    ~16us once per 8 reps).  ScalarE executes nothing but ACTIVATEs —
    no DMA is ever issued on its queue — and the exp/PV/QK chain is
    decoupled by the ST triple-buffer so the PE streams matmuls
    back-to-back at ~216ns/512-col MM.
"""

import math
import sys

for _p in ("/opt/trn_rl_repo",):
    if _p not in sys.path:
        sys.path.insert(0, _p)

import numpy as np

import concourse.bass as bass  # noqa: F401
import concourse.bacc as bacc
import concourse.tile as tile
from concourse import mybir
from concourse.bass_utils import run_bass_kernel_spmd

F32 = mybir.dt.float32
BF16 = mybir.dt.bfloat16
I16 = mybir.dt.int16

B, N, C = 2, 2048, 512
H = 8
D = C // H           # 64
SCALE = float(D) ** -0.5
NT = N // 128        # 16 k-chunks / q-blocks
PAIRS = (B * H) // 8  # 2 (b,h) pairs per core
N_CORES = 8

# (kc, qh) exp tiles that run on the Vector engine (fast-exp) instead of
# ScalarE.  Spread over interior kcs and BOTH q-halves (each q row sees
# 4/16 of its k-terms approximated), alternating engines on each ST
# buffer so neither engine idles across a whole kc.  ScalarE keeps
# 32 - len(DVE_TILES) tiles (pace ~1.17us each vs the PE's ~1.73us/kc).
DVE_TILES = (
    (2, 0), (5, 0), (8, 0), (11, 0), (14, 0),
    (3, 1), (6, 1), (9, 1), (12, 1), (13, 1),
)

# Schraudolph constants: i16 = round(ST*C1 + C2) is the bf16 bit pattern
# of exp(ST*SCALE).  C2's sigma term centers the relative error of the
# linear-in-mantissa approximation (+-3% scallop).
EXP_C1 = float(SCALE * math.log2(math.e) * 128.0)
EXP_C2 = float((127.0 - 0.043) * 128.0)


def build_nc(reps=1, sim_safe=False):
    nc = bacc.Bacc()
    q_in = nc.dram_tensor("q_in", [PAIRS, N, D], F32, kind="ExternalInput")
    k_in = nc.dram_tensor("k_in", [PAIRS, N, D], F32, kind="ExternalInput")
    v_in = nc.dram_tensor("v_in", [PAIRS, N, D], F32, kind="ExternalInput")
    # Output is stored in the DVE 32x32 block-transposed layout
    # out[32*bb + r, t*128 + 32*a + c] = O[q = t*128 + 32*a + r, d = 32*bb + c];
    # unshard_output applies the inverse permute on the host.
    out_t = nc.dram_tensor("out", [PAIRS, D, N], F32, kind="ExternalOutput")

    ins = {"q": q_in, "k": k_in, "v": v_in}

    with tile.TileContext(nc) as tc:
        with (
            tc.tile_pool(name="stat", bufs=1) as stat,
            tc.tile_pool(name="ex", bufs=6) as ex_pool,
            tc.tile_pool(name="st", bufs=3, space="PSUM") as st_pool,
            tc.tile_pool(name="op", bufs=1, space="PSUM") as o_pool,
        ):
            # ---- static cross-slot tiles (explicit buf dim, index j%2) ----
            qstage = stat.tile([128, 2, NT, 128], BF16, tag="qstage")
            kstage = stat.tile([128, 2, NT, 128], BF16, tag="kstage")
            qt = stat.tile([128, 2, NT, 128], BF16, tag="qt")
            kt = stat.tile([128, 2, NT, 128], BF16, tag="kt")
            vt = stat.tile([128, 2, NT, D + 1], BF16, tag="vt")
            ot_tr = stat.tile([96, 2, N], F32, tag="ot_tr")
            o_nm = stat.tile([64, 2, N], F32, tag="o_nm")
            invx = stat.tile([96, 2, 2, 8, 4], F32, tag="invx")

            # one-time init: zero pad columns (never rewritten by loads),
            # ones column of V (denominator accumulator via PV matmul).
            # all on DVE: the gpsimd queue must start the first loads
            # immediately (gpsimd memset is slow and would delay them).
            nc.vector.memset(qstage[:, :, :, D:128], 0.0)
            nc.vector.memset(kstage[:, :, :, D:128], 0.0)
            nc.vector.memset(vt[:, :, :, D : D + 1], 1.0)

            # Warm the ScalarE Exp spline tables before any real work.
            warm = stat.tile([128, 1], F32, tag="warm")
            nc.vector.memset(warm[:], 0.0)
            nc.scalar.activation(
                warm[:], warm[:], mybir.ActivationFunctionType.Exp
            )

            def loads(j):
                b, pair = j % 2, j % PAIRS
                for name, t in ins.items():
                    src = t[pair].rearrange("(t p) d -> p t d", p=128)
                    if name == "v":
                        dst = vt[:, b, :, 0:D]
                    elif name == "q":
                        dst = qstage[:, b, :, 0:D]
                    else:
                        dst = kstage[:, b, :, 0:D]
                    nc.gpsimd.dma_start(out=dst, in_=src)

            def xbars(j):
                b = j % 2
                nc.sync.dma_start_transpose(qt[:, b], qstage[:, b])
                nc.sync.dma_start_transpose(kt[:, b], kstage[:, b])

            # OT~ accumulator: one static [96, 1024] PSUM tile (2 banks)
            # reused by both q-halves of every slot; ST triple-buffer takes
            # the other 6 banks.  A half's first PV (start=True) has a WAR
            # dep on the previous half's epilogue transpose reads.
            ot_ps = o_pool.tile([96, 1024], F32, tag="ot")
            if sim_safe:
                nc.vector.memset(ot_ps[D : 96, :], 0.0)

            def epi_a_half(j, qh):
                # OT~ half -> block-transposed staging ot_tr (DVE 32x32
                # stream transpose straight out of PSUM).  No permute DMAs:
                # normalization and the store happen in this layout; the
                # host-side unshard applies the inverse block permute.
                b = j % 2
                nc.vector.transpose(
                    ot_tr[:, b, qh * 1024 : qh * 1024 + 1024], ot_ps[:]
                )

            def epi_n1(j, qh):
                # reciprocal of the denominator (OT row 64 = ot_tr block
                # row 64..95, inner col 0) and partition-replication of the
                # result to bases 0 and 32 for the two d-block muls.
                b = j % 2
                otv = ot_tr[:, b, qh * 1024 : qh * 1024 + 1024].rearrange(
                    "p (t a c) -> p t a c", t=8, a=4
                )
                nc.vector.reciprocal(
                    invx[64:96, b, qh], otv[64:96, :, :, 0]
                )
                nc.sync.dma_start(out=invx[0:32, b, qh], in_=invx[64:96, b, qh])
                nc.sync.dma_start(out=invx[32:64, b, qh], in_=invx[64:96, b, qh])

            def epi_n2(j, qh):
                # normalize the two real d-blocks (rows 0..63; block row 2
                # holds only the denominator + garbage) and store the half
                # in block-transposed layout.
                b = j % 2
                cols = slice(qh * 1024, qh * 1024 + 1024)
                otv = ot_tr[:, b, cols].rearrange(
                    "p (t a c) -> p t a c", t=8, a=4
                )
                onv = o_nm[:, b, cols].rearrange(
                    "p (t a c) -> p t a c", t=8, a=4
                )
                nc.vector.tensor_mul(
                    onv[:],
                    otv[0:64],
                    invx[0:64, b, qh, :, :, None].broadcast_to([64, 8, 4, 32]),
                )
                nc.sync.dma_start(
                    out=out_t[j % PAIRS][:, cols], in_=o_nm[:, b, cols]
                )

            def compute(j, fills=None):
                b = j % 2
                qtv = qt[:, b].rearrange("d t p -> d (t p)")

                for qh in range(2):
                    def emit_pv(kc, ex):
                        for jj in range(2):
                            nc.tensor.matmul(
                                ot_ps[0 : D + 1, jj * 512 : jj * 512 + 512],
                                vt[:, b, kc, :],
                                ex[:, jj * 512 : jj * 512 + 512],
                                start=(kc == 0),
                                stop=(kc == NT - 1),
                            )

                    prevs = []
                    for kc in range(NT):
                        ex = ex_pool.tile([128, 1024], BF16, tag="ex")
                        st = st_pool.tile([128, 1024], F32, tag="st")
                        for jj in range(2):
                            q0 = qh * 1024 + jj * 512
                            nc.tensor.matmul(
                                st[:, jj * 512 : jj * 512 + 512],
                                kt[:, b, kc, :],
                                qtv[:, q0 : q0 + 512],
                                start=True,
                                stop=True,
                            )
                        if (kc, qh) in DVE_TILES:
                            nc.vector.tensor_scalar(
                                out=ex.bitcast(I16)[:],
                                in0=st[:],
                                scalar1=EXP_C1,
                                scalar2=EXP_C2,
                                op0=mybir.AluOpType.mult,
                                op1=mybir.AluOpType.add,
                            )
                        else:
                            nc.scalar.activation(
                                ex[:],
                                st[:],
                                mybir.ActivationFunctionType.Exp,
                                scale=SCALE,
                            )
                        if fills and (qh, kc) in fills:
                            for fn in fills[(qh, kc)]:
                                fn()
                        prevs.append(ex)
                        # PV lags exp by TWO kcs so its moving operand is
                        # always two exp-service periods old — the PE never
                        # waits out an exp tail + semaphore propagation.
                        if kc >= 2:
                            emit_pv(kc - 2, prevs[kc - 2])
                    emit_pv(NT - 2, prevs[NT - 2])
                    emit_pv(NT - 1, prevs[NT - 1])
                    if qh == 0:
                        epi_a_half(j, 0)

            def emit_slot(j, S, wrap, do_epi_a=True, do_epi_b=True):
                """Emit slot j of a stream of S slots. wrap=True means
                out-of-range lookahead/lag indices wrap mod S (loop body);
                wrap=False skips out-of-range work (straight-line path and
                the peeled first body via do_epi_a/do_epi_b)."""
                jl = (j + 2) % S if wrap else j + 2
                jx = (j + 1) % S if wrap else j + 1
                if wrap or jl < S:
                    loads(jl)
                if wrap or jx < S:
                    xbars(jx)
                # epi-A(j-1, half1) must run before compute(j)'s first PV
                # (it reads the shared OT PSUM banks); its inputs are ready
                # at the slot boundary so it doesn't stall the DVE stream.
                if do_epi_a and (wrap or j - 1 >= 0):
                    epi_a_half((j - 1) % S, 1)
                # The normalize+store of each transposed half rides as fills
                # a few kcs later, where the in-order DVE stream reaches it
                # with all inputs long since ready.
                fills = {}
                if do_epi_b and (wrap or j - 1 >= 0):
                    jp = (j - 1) % S
                    fills[(0, 2)] = [lambda: epi_n1(jp, 1)]
                    fills[(0, 6)] = [lambda: epi_n2(jp, 1)]
                fills[(1, 2)] = [lambda: epi_n1(j % S, 0)]
                fills[(1, 6)] = [lambda: epi_n2(j % S, 0)]
                compute(j, fills=fills)

            if reps == 1:
                S = PAIRS
                loads(0)
                loads(1)
                xbars(0)
                emit_slot(0, S, wrap=False)
                emit_slot(1, S, wrap=False)
                epi_a_half(1, 1)
                epi_n1(1, 1)
                epi_n2(1, 1)
            else:
                assert reps % 16 == 0, "reps must be 1 or a multiple of 16"
                S = 16 * PAIRS  # 32 slots per body
                loads(0)
                loads(1)
                xbars(0)
                # first body is peeled: its slots 0/1 have no prior slot
                # to lag behind, and its tail slots prefetch the loop's
                # first iteration.  For_i runs the rest.
                for j in range(S):
                    emit_slot(j, S, wrap=True,
                              do_epi_a=(j >= 1), do_epi_b=(j >= 2))
                with tc.For_i(0, reps // 16 - 1, 1):
                    for j in range(S):
                        emit_slot(j, S, wrap=True)
                epi_a_half(S - 1, 1)
                epi_n1(S - 1, 1)
                epi_n2(S - 1, 1)

    nc.compile()
    return nc


def shard_inputs(query, key, value):
    """[B, N, C] -> per-core dicts of [PAIRS, N, D] slices."""
    def to_pairs(x):
        return np.ascontiguousarray(
            x.reshape(B, N, H, D).transpose(0, 2, 1, 3).reshape(B * H, N, D)
        )

    qp, kp, vp = to_pairs(query), to_pairs(key), to_pairs(value)
    in_maps = []
    for c in range(N_CORES):
        s = slice(c * PAIRS, (c + 1) * PAIRS)
        in_maps.append({"q_in": qp[s], "k_in": kp[s], "v_in": vp[s]})
    return in_maps


def unshard_output(results):
    """per-core block-transposed [PAIRS, D, N] -> [B, N, C].

    Device layout: out[pair, 32*bb + r, t*128 + 32*a + c] =
    O[pair, q = t*128 + 32*a + r, d = 32*bb + c].
    """
    outs = np.concatenate([results[c]["out"] for c in range(N_CORES)], axis=0)
    arr = outs.reshape(B * H, 2, 32, NT, 4, 32)       # pair, bb, r, t, a, c
    o = arr.transpose(0, 3, 4, 2, 1, 5).reshape(B * H, N, D)  # pair, q, d
    return np.ascontiguousarray(
        o.reshape(B, H, N, D).transpose(0, 2, 1, 3).reshape(B, N, C)
    )


def kernel(query, key, value):
    query = np.asarray(query, dtype=np.float32)
    key = np.asarray(key, dtype=np.float32)
    value = np.asarray(value, dtype=np.float32)
    nc = build_nc()
    in_maps = shard_inputs(query, key, value)
    res = run_bass_kernel_spmd(nc, in_maps, core_ids=list(range(N_CORES)))
    return unshard_output(res.results)


# revision 41
# speedup vs baseline: 1.0946x; 1.0791x over previous
"""Multi-head attention kernel for Trainium2 (Bass/Tile), 8 NeuronCores.

Problem: B=2, N=2048, C=512, H=8 heads, D=64. softmax(Q K^T / sqrt(D)) V.

Sharding: the 16 (batch, head) pairs are split 2-per-core across 8 cores
(data + head parallel, no communication).

Design (v2; ~74us/rep vs the ~104us v1 baseline):

  - All matmuls in bf16.  Q, K, V are loaded from HBM with gpsimd
    cast-DMAs (fp32 -> bf16 conversion is free in the DMA datapath) into
    [128, NT, 128] staging tiles whose columns 64..127 are zeroed once;
    a single DMA-xbar transpose (`dma_start_transpose`, 16x128 tiles)
    per tensor then produces Q^T / K^T in the canonical
    [d(64)+zero-pad(64), (t, p)] layout the PE wants — no DVE
    stream-transposes or block-permute DMAs in the prologue, and the
    128-partition zero pad (full SBUF->PE stream bandwidth for the
    moving operand) falls out of the xbar's row = ell % 128 mapping.

  - Per (b, h) pair ("slot"), per q-half (1024 queries), per k-chunk kc
    (16 chunks of 128 keys):
      ST[kc] = K_T[kc].T @ Q_T_half  -> [128k, 1024q] fp32 PSUM
      expST:  most (kc, qh) tiles on ScalarE (activation Exp, bf16 out);
      the 8 DVE_TILES run on the Vector engine as ONE tensor_scalar:
      i16 = round(ST*C1 + C2) written through an int16 bitcast of the
      bf16 ex tile IS the bf16 bit pattern of exp(ST*scale) (Schraudolph;
      convert-on-write rounds to nearest; +-3% scallop, tuned sigma).
      This splits exp across two engines (ScalarE alone would be the
      bottleneck at ~73us/rep).
      OT~ [65, 1024q] += [V[kc] | 1].T @ expST  (row 64 = denominator).
    PSUM: ST triple-buffered (3 x 2 banks) + one OT (2 banks) = 8 banks.
  - Epilogue per half: DVE 32x32 stream-transpose of OT~ out of PSUM,
    then (lagged, as fills inside later compute) a reciprocal of the
    denominator positions, two tiny partition-replication DMAs, and two
    32-partition broadcast multiplies — all IN the block-transposed
    layout.  The half is stored to HBM in that layout and
    unshard_output applies the inverse 32x32 block permute on the host,
    eliminating the 24 block-permute DMAs per slot that previously
    congested the sync/gpsimd queues and stalled the DVE stream.

  - Schedule: a flat stream of slots, 16 per unrolled For_i body
    (= 8 reps) plus a peeled first body; reps==1 is a 2-slot
    straight-line version of the same stream.  Slot j emits:
      loads(j+2)   [gpsimd cast-DMAs]
      xbars(j+1)   [sync HWDGE, async on the DMA engines]
      epi-A(j-1, half1)  [must precede slot j's first PV: reads the
                          shared OT PSUM banks; ready at the boundary]
      compute(j)   [QK -> exp -> lagged PV per kc; epi-A(j, half0)
                    inline at the half boundary; normalize+store of
                    older halves emitted as fills at kc2/kc6 so the
                    in-order DVE stream reaches each epilogue op long
                    after its inputs landed]
    Cross-slot tiles are static with an explicit j%2 buffer dim, so the
    lookahead survives the For_i back edge (the all-engine barrier costs
    ~16us once per 8 reps).  ScalarE executes nothing but ACTIVATEs —
    no DMA is ever issued on its queue — and the exp/PV/QK chain is
    decoupled by the ST triple-buffer so the PE streams matmuls
    back-to-back at ~216ns/512-col MM.
"""

import math
import sys

for _p in ("/opt/trn_rl_repo",):
    if _p not in sys.path:
        sys.path.insert(0, _p)

import numpy as np

import concourse.bass as bass  # noqa: F401
import concourse.bacc as bacc
import concourse.tile as tile
from concourse import mybir
from concourse.bass_utils import run_bass_kernel_spmd

F32 = mybir.dt.float32
BF16 = mybir.dt.bfloat16
I16 = mybir.dt.int16

B, N, C = 2, 2048, 512
H = 8
D = C // H           # 64
SCALE = float(D) ** -0.5
NT = N // 128        # 16 k-chunks / q-blocks
PAIRS = (B * H) // 8  # 2 (b,h) pairs per core
N_CORES = 8

# (kc, qh) exp tiles that run on the Vector engine (fast-exp) instead of
# ScalarE.  Spread over interior kcs and BOTH q-halves (each q row sees
# 4/16 of its k-terms approximated), alternating engines on each ST
# buffer so neither engine idles across a whole kc.  ScalarE keeps
# 32 - len(DVE_TILES) tiles (pace ~1.17us each vs the PE's ~1.73us/kc).
DVE_TILES = (
    (2, 0), (5, 0), (8, 0), (11, 0), (14, 0),
    (3, 1), (6, 1), (9, 1), (12, 1), (13, 1),
)

# Schraudolph constants: i16 = round(ST*C1 + C2) is the bf16 bit pattern
# of exp(ST*SCALE).  C2's sigma term centers the relative error of the
# linear-in-mantissa approximation (+-3% scallop).
EXP_C1 = float(SCALE * math.log2(math.e) * 128.0)
EXP_C2 = float((127.0 - 0.043) * 128.0)


def build_nc(reps=1, sim_safe=False):
    nc = bacc.Bacc()
    q_in = nc.dram_tensor("q_in", [PAIRS, N, D], F32, kind="ExternalInput")
    k_in = nc.dram_tensor("k_in", [PAIRS, N, D], F32, kind="ExternalInput")
    v_in = nc.dram_tensor("v_in", [PAIRS, N, D], F32, kind="ExternalInput")
    # Output is stored in the DVE 32x32 block-transposed layout
    # out[32*bb + r, t*128 + 32*a + c] = O[q = t*128 + 32*a + r, d = 32*bb + c];
    # unshard_output applies the inverse permute on the host.
    out_t = nc.dram_tensor("out", [PAIRS, D, N], F32, kind="ExternalOutput")

    ins = {"q": q_in, "k": k_in, "v": v_in}

    with tile.TileContext(nc) as tc:
        with (
            tc.tile_pool(name="stat", bufs=1) as stat,
            tc.tile_pool(name="ex", bufs=6) as ex_pool,
            tc.tile_pool(name="st", bufs=3, space="PSUM") as st_pool,
            tc.tile_pool(name="op", bufs=1, space="PSUM") as o_pool,
        ):
            # ---- static cross-slot tiles (explicit buf dim, index j%2) ----
            # Q/K staging as SEPARATE per-buffer tiles (not one tile with
            # a buf dim): region tracking for DMA writes is whole-tile
            # coarse, so a single tile falsely serializes slot j+1's load
            # behind slot j's xbar transpose read.
            qstage0 = stat.tile([128, NT, 128], BF16, tag="qstage0")
            qstage1 = stat.tile([128, NT, 128], BF16, tag="qstage1")
            kstage0 = stat.tile([128, NT, 128], BF16, tag="kstage0")
            kstage1 = stat.tile([128, NT, 128], BF16, tag="kstage1")
            qstages = [qstage0, qstage1]
            kstages = [kstage0, kstage1]
            qt = stat.tile([128, 2, NT, 128], BF16, tag="qt")
            kt = stat.tile([128, 2, NT, 128], BF16, tag="kt")
            vt = stat.tile([128, 2, NT, D + 1], BF16, tag="vt")
            ot_tr = stat.tile([96, 2, N], F32, tag="ot_tr")
            o_nm = stat.tile([64, 2, N], F32, tag="o_nm")
            invx = stat.tile([96, 2, 2, 8, 4], F32, tag="invx")

            # one-time init: zero pad columns (never rewritten by loads),
            # ones column of V (denominator accumulator via PV matmul).
            # all on DVE: the gpsimd queue must start the first loads
            # immediately (gpsimd memset is slow and would delay them).
            for _s in (qstage0, qstage1, kstage0, kstage1):
                nc.vector.memset(_s[:, :, D:128], 0.0)
            nc.vector.memset(vt[:, :, :, D : D + 1], 1.0)

            # Warm the ScalarE Exp spline tables before any real work.
            warm = stat.tile([128, 1], F32, tag="warm")
            nc.vector.memset(warm[:], 0.0)
            nc.scalar.activation(
                warm[:], warm[:], mybir.ActivationFunctionType.Exp
            )

            def loads(j):
                b, pair = j % 2, j % PAIRS
                for name, t in ins.items():
                    src = t[pair].rearrange("(t p) d -> p t d", p=128)
                    if name == "v":
                        dst = vt[:, b, :, 0:D]
                    elif name == "q":
                        dst = qstages[b][:, :, 0:D]
                    else:
                        dst = kstages[b][:, :, 0:D]
                    nc.gpsimd.dma_start(out=dst, in_=src)

            def xbars(j):
                b = j % 2
                nc.sync.dma_start_transpose(qt[:, b], qstages[b][:])
                nc.sync.dma_start_transpose(kt[:, b], kstages[b][:])

            # OT~ accumulator: one static [96, 1024] PSUM tile (2 banks)
            # reused by both q-halves of every slot; ST triple-buffer takes
            # the other 6 banks.  A half's first PV (start=True) has a WAR
            # dep on the previous half's epilogue transpose reads.
            ot_ps = o_pool.tile([96, 1024], F32, tag="ot")
            if sim_safe:
                nc.vector.memset(ot_ps[D : 96, :], 0.0)

            def epi_a_half(j, qh):
                # OT~ half -> block-transposed staging ot_tr (DVE 32x32
                # stream transpose straight out of PSUM).  No permute DMAs:
                # normalization and the store happen in this layout; the
                # host-side unshard applies the inverse block permute.
                b = j % 2
                nc.vector.transpose(
                    ot_tr[:, b, qh * 1024 : qh * 1024 + 1024], ot_ps[:]
                )

            def epi_n1(j, qh):
                # reciprocal of the denominator (OT row 64 = ot_tr block
                # row 64..95, inner col 0) and partition-replication of the
                # result to bases 0 and 32 for the two d-block muls.
                b = j % 2
                otv = ot_tr[:, b, qh * 1024 : qh * 1024 + 1024].rearrange(
                    "p (t a c) -> p t a c", t=8, a=4
                )
                nc.vector.reciprocal(
                    invx[64:96, b, qh], otv[64:96, :, :, 0]
                )
                nc.sync.dma_start(out=invx[0:32, b, qh], in_=invx[64:96, b, qh])
                nc.sync.dma_start(out=invx[32:64, b, qh], in_=invx[64:96, b, qh])

            def epi_n2(j, qh):
                # normalize the two real d-blocks (rows 0..63; block row 2
                # holds only the denominator + garbage) and store the half
                # in block-transposed layout.
                b = j % 2
                cols = slice(qh * 1024, qh * 1024 + 1024)
                otv = ot_tr[:, b, cols].rearrange(
                    "p (t a c) -> p t a c", t=8, a=4
                )
                onv = o_nm[:, b, cols].rearrange(
                    "p (t a c) -> p t a c", t=8, a=4
                )
                nc.vector.tensor_mul(
                    onv[:],
                    otv[0:64],
                    invx[0:64, b, qh, :, :, None].broadcast_to([64, 8, 4, 32]),
                )
                nc.sync.dma_start(
                    out=out_t[j % PAIRS][:, cols], in_=o_nm[:, b, cols]
                )

            def compute(j, fills=None):
                b = j % 2
                qtv = qt[:, b].rearrange("d t p -> d (t p)")

                for qh in range(2):
                    def emit_pv(kc, ex):
                        for jj in range(2):
                            nc.tensor.matmul(
                                ot_ps[0 : D + 1, jj * 512 : jj * 512 + 512],
                                vt[:, b, kc, :],
                                ex[:, jj * 512 : jj * 512 + 512],
                                start=(kc == 0),
                                stop=(kc == NT - 1),
                            )

                    prevs = []
                    for kc in range(NT):
                        ex = ex_pool.tile([128, 1024], BF16, tag="ex")
                        st = st_pool.tile([128, 1024], F32, tag="st")
                        for jj in range(2):
                            q0 = qh * 1024 + jj * 512
                            nc.tensor.matmul(
                                st[:, jj * 512 : jj * 512 + 512],
                                kt[:, b, kc, :],
                                qtv[:, q0 : q0 + 512],
                                start=True,
                                stop=True,
                            )
                        if (kc, qh) in DVE_TILES:
                            nc.vector.tensor_scalar(
                                out=ex.bitcast(I16)[:],
                                in0=st[:],
                                scalar1=EXP_C1,
                                scalar2=EXP_C2,
                                op0=mybir.AluOpType.mult,
                                op1=mybir.AluOpType.add,
                            )
                        else:
                            nc.scalar.activation(
                                ex[:],
                                st[:],
                                mybir.ActivationFunctionType.Exp,
                                scale=SCALE,
                            )
                        if fills and (qh, kc) in fills:
                            for fn in fills[(qh, kc)]:
                                fn()
                        prevs.append(ex)
                        # PV lags exp by TWO kcs so its moving operand is
                        # always two exp-service periods old — the PE never
                        # waits out an exp tail + semaphore propagation.
                        if kc >= 2:
                            emit_pv(kc - 2, prevs[kc - 2])
                    emit_pv(NT - 2, prevs[NT - 2])
                    emit_pv(NT - 1, prevs[NT - 1])
                    if qh == 0:
                        epi_a_half(j, 0)

            def emit_slot(j, S, wrap, do_epi_a=True, do_epi_b=True):
                """Emit slot j of a stream of S slots. wrap=True means
                out-of-range lookahead/lag indices wrap mod S (loop body);
                wrap=False skips out-of-range work (straight-line path and
                the peeled first body via do_epi_a/do_epi_b)."""
                jl = (j + 2) % S if wrap else j + 2
                jx = (j + 1) % S if wrap else j + 1
                if wrap or jl < S:
                    loads(jl)
                if wrap or jx < S:
                    xbars(jx)
                # epi-A(j-1, half1) must run before compute(j)'s first PV
                # (it reads the shared OT PSUM banks); its inputs are ready
                # at the slot boundary so it doesn't stall the DVE stream.
                if do_epi_a and (wrap or j - 1 >= 0):
                    epi_a_half((j - 1) % S, 1)
                # The normalize+store of each transposed half rides as fills
                # a few kcs later, where the in-order DVE stream reaches it
                # with all inputs long since ready.
                fills = {}
                if do_epi_b and (wrap or j - 1 >= 0):
                    jp = (j - 1) % S
                    fills[(0, 2)] = [lambda: epi_n1(jp, 1)]
                    fills[(0, 6)] = [lambda: epi_n2(jp, 1)]
                fills[(1, 2)] = [lambda: epi_n1(j % S, 0)]
                fills[(1, 6)] = [lambda: epi_n2(j % S, 0)]
                compute(j, fills=fills)

            if reps == 1:
                S = PAIRS
                loads(0)
                loads(1)
                xbars(0)
                emit_slot(0, S, wrap=False)
                emit_slot(1, S, wrap=False)
                epi_a_half(1, 1)
                epi_n1(1, 1)
                epi_n2(1, 1)
            else:
                assert reps % 16 == 0, "reps must be 1 or a multiple of 16"
                S = 16 * PAIRS  # 32 slots per body
                loads(0)
                loads(1)
                xbars(0)
                # first body is peeled: its slots 0/1 have no prior slot
                # to lag behind, and its tail slots prefetch the loop's
                # first iteration.  For_i runs the rest.
                for j in range(S):
                    emit_slot(j, S, wrap=True,
                              do_epi_a=(j >= 1), do_epi_b=(j >= 2))
                with tc.For_i(0, reps // 16 - 1, 1):
                    for j in range(S):
                        emit_slot(j, S, wrap=True)
                epi_a_half(S - 1, 1)
                epi_n1(S - 1, 1)
                epi_n2(S - 1, 1)

    nc.compile()
    return nc


def shard_inputs(query, key, value):
    """[B, N, C] -> per-core dicts of [PAIRS, N, D] slices."""
    def to_pairs(x):
        return np.ascontiguousarray(
            x.reshape(B, N, H, D).transpose(0, 2, 1, 3).reshape(B * H, N, D)
        )

    qp, kp, vp = to_pairs(query), to_pairs(key), to_pairs(value)
    in_maps = []
    for c in range(N_CORES):
        s = slice(c * PAIRS, (c + 1) * PAIRS)
        in_maps.append({"q_in": qp[s], "k_in": kp[s], "v_in": vp[s]})
    return in_maps


def unshard_output(results):
    """per-core block-transposed [PAIRS, D, N] -> [B, N, C].

    Device layout: out[pair, 32*bb + r, t*128 + 32*a + c] =
    O[pair, q = t*128 + 32*a + r, d = 32*bb + c].
    """
    outs = np.concatenate([results[c]["out"] for c in range(N_CORES)], axis=0)
    arr = outs.reshape(B * H, 2, 32, NT, 4, 32)       # pair, bb, r, t, a, c
    o = arr.transpose(0, 3, 4, 2, 1, 5).reshape(B * H, N, D)  # pair, q, d
    return np.ascontiguousarray(
        o.reshape(B, H, N, D).transpose(0, 2, 1, 3).reshape(B, N, C)
    )


def kernel(query, key, value):
    query = np.asarray(query, dtype=np.float32)
    key = np.asarray(key, dtype=np.float32)
    value = np.asarray(value, dtype=np.float32)
    nc = build_nc()
    in_maps = shard_inputs(query, key, value)
    res = run_bass_kernel_spmd(nc, in_maps, core_ids=list(range(N_CORES)))
    return unshard_output(res.results)
